# revision 13
# baseline (speedup 1.0000x reference)
"""DKVMN kernel for Trainium2 (8 NeuronCores, data-parallel over batch).

Shapes (hardcoded): B=64, S=200, INUM=1000, IN_DIM=2000, CNUM=50, EDIM=128.
Per core: B_loc = 8 batches. All engines balanced via two identities:

kappa-substitution: with er = 1/e, kap_t = a_t*er_t, the state y = v - kappa
(kappa_{t-1} := kap_t) follows y_t = (1 - w_t e_t) y_{t-1} + d1_t where
d1_t = kap_t - kap_{t+1} is c-INDEPENDENT -> the scan's additive input is a
shared row; the old A = W*a bulk pass disappears.

Delta-U identity: softmax weights sum to 1 over c, so the read
r_t = sum_c w_t[c] v_{t-1}[c,:] = (U_{t-1} - U_t + a_t) * er_t with
U_t = sum_c v_t = Uy_t + C*kap_{t+1}. So r = (Uy_{t-1}-Uy_t)*er + f2,
f2 = (C*d1)*er + kap. Uy = sum_c y comes from 50 identity-stationary
accumulating PE matmuls (PSUM) -> the old X = W*V pass and the 50-matmul
lin1 reduction disappear.

Per b: w-broadcast DMA -> WGM tile; G = WGM*e_bv (TT, DVE or Pool, in-place);
M = 1-G (ACT affine copy, in-place); 50 per-c scans (DVE, initial=y_init AP);
50 ident matmuls -> Uy (PE); r smalls (DVE); per pair: hps = lin1@r16 +
lin2@itm; h = tanh; out = sigmoid(h@cls_w) in bf16, host upcasts to f32.
"""

import numpy as np
import ml_dtypes

import concourse.bass as bass
import concourse.mybir as mybir
import concourse.tile as tile
from concourse import bacc
from concourse.bass_utils import run_bass_kernel_spmd

F32 = mybir.dt.float32
BF16 = mybir.dt.bfloat16
FP16 = mybir.dt.float16
AF = mybir.ActivationFunctionType
OP = mybir.AluOpType

B, S, INUM, IN_DIM, CNUM, EDIM = 64, 200, 1000, 2000, 50, 128
NCORES = 8
BL = B // NCORES          # 8 batches per core
BT = BL * S               # 1600
IK = 8                    # INUM k-chunks of 125
DK = 16                   # IN_DIM k-chunks of 125
KC = 125
CT = CNUM * S             # 10000
SP = S + 1

# per-b split of the G = W*e multiply: c < G_SPLIT[b] on Pool, rest on
# DVE. Last batch all-DVE to shorten the tail.
G_SPLIT = [44, 44, 44, 44, 44, 44, 44, 44]

_NC_CACHE = {}
LAST_RESULT = None


def _build():
    nc = bacc.Bacc("TRN2", target_bir_lowering=False, debug=False,
                   num_devices=NCORES)

    itemT = nc.dram_tensor("itemT", [INUM, BT], BF16, kind="ExternalInput")
    interT = nc.dram_tensor("interT", [IN_DIM, BT], BF16, kind="ExternalInput")
    A_wT = nc.dram_tensor("A_wT", [INUM, EDIM], BF16, kind="ExternalInput")
    B_wT = nc.dram_tensor("B_wT", [IN_DIM, EDIM], BF16, kind="ExternalInput")
    kmatT = nc.dram_tensor("kmatT", [EDIM, CNUM], BF16, kind="ExternalInput")
    er_wT = nc.dram_tensor("er_wT", [EDIM, EDIM], BF16, kind="ExternalInput")
    ad_wT = nc.dram_tensor("ad_wT", [EDIM, EDIM], BF16, kind="ExternalInput")
    lin1T = nc.dram_tensor("lin1T", [EDIM, EDIM], BF16, kind="ExternalInput")
    lin2T = nc.dram_tensor("lin2T", [EDIM, EDIM], BF16, kind="ExternalInput")
    cls_wT = nc.dram_tensor("cls_wT", [EDIM, INUM], BF16, kind="ExternalInput")
    v0f = nc.dram_tensor("v0f", [EDIM, CNUM], F32, kind="ExternalInput")
    idm = nc.dram_tensor("idm", [EDIM, EDIM], BF16, kind="ExternalInput")
    lin_b = nc.dram_tensor("lin_b", [EDIM], F32, kind="ExternalInput")
    er_b = nc.dram_tensor("er_b", [EDIM], F32, kind="ExternalInput")
    ad_b = nc.dram_tensor("ad_b", [EDIM], F32, kind="ExternalInput")
    cls_b16 = nc.dram_tensor("cls_b16", [1, INUM], BF16, kind="ExternalInput")
    out = nc.dram_tensor("out", [BT, INUM], BF16, kind="ExternalOutput")
    w_r = nc.dram_tensor("w_r", [BL, CNUM, S], FP16, kind="Internal")

    with tile.TileContext(nc) as tc:
        with tc.tile_pool(name="singles", bufs=1) as sg:
            ones16 = sg.tile([1, 128], BF16, tag="ones16")
            nc.vector.memset(ones16[:], 1.0)
            ones50 = sg.tile([CNUM, 1], BF16, tag="ones50")
            nc.vector.memset(ones50[:], 1.0)

            A_w_sb = sg.tile([KC, IK, EDIM], BF16, tag="A_w_sb")
            B_w_sb = sg.tile([KC, DK, EDIM], BF16, tag="B_w_sb")
            kmat_sb = sg.tile([EDIM, CNUM], BF16, tag="kmat_sb")
            er_w_sb = sg.tile([EDIM, EDIM], BF16, tag="er_w_sb")
            ad_w_sb = sg.tile([EDIM, EDIM], BF16, tag="ad_w_sb")
            lin1_sb = sg.tile([EDIM, EDIM], BF16, tag="lin1_sb")
            lin2_sb = sg.tile([EDIM, EDIM], BF16, tag="lin2_sb")
            cls_w_sb = sg.tile([EDIM, INUM], BF16, tag="cls_w_sb")
            v0_sb = sg.tile([EDIM, CNUM], F32, tag="v0_sb")
            id_sb = sg.tile([EDIM, EDIM], BF16, tag="id_sb")
            lin_b_col = sg.tile([EDIM, 1], F32, tag="lin_b_col")
            er_b_col = sg.tile([EDIM, 1], F32, tag="er_b_col")
            ad_b_col = sg.tile([EDIM, 1], F32, tag="ad_b_col")
            cls_b_sb = sg.tile([1, INUM], BF16, tag="cls_b_sb")

            # softmax-critical weights first so chunk 0 starts ASAP
            nc.sync.dma_start(
                A_w_sb[:], A_wT.ap().rearrange("(k p) e -> p k e", p=KC))
            nc.sync.dma_start(kmat_sb[:], kmatT.ap())
            nc.sync.dma_start(
                B_w_sb[:], B_wT.ap().rearrange("(k p) e -> p k e", p=KC))
            nc.sync.dma_start(er_w_sb[:], er_wT.ap())
            nc.sync.dma_start(ad_w_sb[:], ad_wT.ap())
            nc.sync.dma_start(er_b_col[:], er_b.ap()[:, None])
            nc.sync.dma_start(ad_b_col[:], ad_b.ap()[:, None])
            nc.sync.dma_start(v0_sb[:], v0f.ap())
            nc.sync.dma_start(id_sb[:], idm.ap())
            nc.sync.dma_start(lin1_sb[:], lin1T.ap())
            nc.sync.dma_start(lin2_sb[:], lin2T.ap())
            nc.sync.dma_start(cls_w_sb[:], cls_wT.ap())
            nc.sync.dma_start(lin_b_col[:], lin_b.ap()[:, None])
            nc.sync.dma_start(cls_b_sb[:], cls_b16.ap())

            # persistent per-core activations / smalls
            itm16 = sg.tile([EDIM, BT], BF16, tag="itm16")
            e16 = sg.tile([EDIM, BL, S], BF16, tag="e16")
            a16 = sg.tile([EDIM, BL, S], BF16, tag="a16")
            er = sg.tile([EDIM, BL, S], BF16, tag="er")
            kap = sg.tile([EDIM, BL, SP], BF16, tag="kap")
            d1t = sg.tile([EDIM, BL, S], FP16, tag="d1t")
            f2 = sg.tile([EDIM, BL, S], BF16, tag="f2")
            Ub = sg.tile([EDIM, BL, SP], F32, tag="Ub")
            yinit = sg.tile([EDIM, BL, CNUM], F32, tag="yinit")
            r16 = sg.tile([EDIM, BL, S], BF16, tag="r16")
            Sv0 = sg.tile([EDIM, 1], F32, tag="Sv0")
            rtmp = sg.tile([EDIM, 2, S], F32, tag="rtmp")

            nc.vector.memset(kap[:, :, S:SP], 0.0)
            nc.vector.tensor_reduce(out=Sv0[:], in_=v0_sb[:], op=OP.add,
                                    axis=mybir.AxisListType.X)

            with tc.tile_pool(name="p1", bufs=2) as p1, \
                 tc.tile_pool(name="p1w", bufs=2) as p1w, \
                 tc.tile_pool(name="p1ps", bufs=2, space="PSUM") as p1p, \
                 tc.tile_pool(name="wgm", bufs=3) as wgmp, \
                 tc.tile_pool(name="vp", bufs=2) as vp, \
                 tc.tile_pool(name="ups", bufs=2, space="PSUM") as upsp, \
                 tc.tile_pool(name="hps", bufs=2, space="PSUM") as hpsp, \
                 tc.tile_pool(name="clsps", bufs=2, space="PSUM") as clsp, \
                 tc.tile_pool(name="p3", bufs=2) as p3:
                def phase1(q):
                    c0 = q * 400
                    cols = slice(c0, c0 + 400)
                    pr = slice(2 * q, 2 * q + 2)

                    # ---- phase 1, chunk q (2 batches) ----
                    it_ch = p1.tile([KC, IK, 400], BF16, tag="it_ch")
                    nc.sync.dma_start(
                        it_ch[:], itemT.ap()[:, cols].rearrange(
                            "(k p) t -> p k t", p=KC))
                    in_ch = p1.tile([KC, DK, 400], BF16, tag="in_ch")
                    nc.sync.dma_start(
                        in_ch[:], interT.ap()[:, cols].rearrange(
                            "(k p) t -> p k t", p=KC))

                    ps1 = p1p.tile([EDIM, 400], F32, tag="big")
                    for k in range(IK):
                        nc.tensor.matmul(ps1[:], A_w_sb[:, k, :],
                                         it_ch[:, k, :],
                                         start=(k == 0), stop=(k == IK - 1))
                    nc.scalar.copy(itm16[:, cols], ps1[:])

                    # softmax over memory slots -> w_r (fp16)
                    ps2 = p1p.tile([EDIM, 400], F32, tag="big")
                    nc.tensor.matmul(ps2[:CNUM], kmat_sb[:], itm16[:, cols],
                                     start=True, stop=True)
                    E_j = p1w.tile([CNUM, 400], BF16, tag="E_j")
                    nc.scalar.activation(E_j[:], ps2[:CNUM], AF.Exp)
                    ps2b = p1p.tile([EDIM, 400], F32, tag="big")
                    nc.tensor.matmul(ps2b[0:1], ones50[:], E_j[:],
                                     start=True, stop=True)
                    zr = p1w.tile([1, 400], F32, tag="zr")
                    nc.vector.reciprocal(zr[:], ps2b[0:1])
                    zr16 = p1w.tile([1, 400], BF16, tag="zr16")
                    nc.scalar.copy(zr16[:], zr[:])
                    ps2c = p1p.tile([EDIM, 400], F32, tag="big")
                    nc.tensor.matmul(ps2c[:CNUM], ones16[:1, :CNUM], zr16[:],
                                     start=True, stop=True)
                    w_j = p1w.tile([CNUM, 400], FP16, tag="w_j")
                    nc.vector.scalar_tensor_tensor(
                        out=w_j[:], in0=E_j[:], scalar=1.0,
                        in1=ps2c[:CNUM], op0=OP.mult, op1=OP.mult)
                    nc.sync.dma_start(w_r.ap()[2 * q], w_j[:, 0:S])
                    nc.sync.dma_start(w_r.ap()[2 * q + 1], w_j[:, S:2 * S])

                    ps3 = p1p.tile([EDIM, 400], F32, tag="big")
                    for k in range(DK):
                        nc.tensor.matmul(ps3[:], B_w_sb[:, k, :],
                                         in_ch[:, k, :],
                                         start=(k == 0), stop=(k == DK - 1))
                    itr_j = p1w.tile([EDIM, 400], BF16, tag="itr_j")
                    nc.scalar.copy(itr_j[:], ps3[:])

                    ps4 = p1p.tile([EDIM, 400], F32, tag="big")
                    nc.tensor.matmul(ps4[:], er_w_sb[:], itr_j[:],
                                     start=True, stop=True)
                    nc.scalar.activation(e16[:, pr, :], ps4[:],
                                         AF.Sigmoid, bias=er_b_col[:],
                                         scale=1.0)
                    ps5 = p1p.tile([EDIM, 400], F32, tag="big")
                    nc.tensor.matmul(ps5[:], ad_w_sb[:], itr_j[:],
                                     start=True, stop=True)
                    nc.scalar.activation(a16[:, pr, :], ps5[:],
                                         AF.Tanh, bias=ad_b_col[:], scale=1.0)

                    # kappa smalls: er = 1/e ; kap = a*er ; d1 = kap - kap_+1
                    # f2 = (C*d1)*er + kap
                    with nc.allow_low_precision(reason="er=1/e in bf16 is used self-consistently"):
                        nc.vector.reciprocal(er[:, pr, :], e16[:, pr, :])
                    nc.gpsimd.tensor_tensor(out=kap[:, pr, 0:S],
                                            in0=a16[:, pr, :],
                                            in1=er[:, pr, :], op=OP.mult)
                    nc.gpsimd.tensor_tensor(out=d1t[:, pr, :],
                                            in0=kap[:, pr, 0:S],
                                            in1=kap[:, pr, 1:SP],
                                            op=OP.subtract)
                    nc.vector.scalar_tensor_tensor(
                        out=f2[:, pr, :], in0=d1t[:, pr, :],
                        scalar=float(CNUM), in1=er[:, pr, :],
                        op0=OP.mult, op1=OP.mult)
                    nc.gpsimd.tensor_tensor(out=f2[:, pr, :],
                                            in0=f2[:, pr, :],
                                            in1=kap[:, pr, 0:S], op=OP.add)

                    for b in (2 * q, 2 * q + 1):
                        nc.gpsimd.tensor_tensor(
                            out=yinit[:, b, :], in0=v0_sb[:],
                            in1=kap[:, b, 0:1].to_broadcast([EDIM, CNUM]),
                            op=OP.subtract)
                        nc.vector.scalar_tensor_tensor(
                            out=Ub[:, b, 0:1], in0=kap[:, b, 0:1],
                            scalar=-float(CNUM), in1=Sv0[:],
                            op0=OP.mult, op1=OP.add)

                def phase23(b):
                    if True:
                        bc0 = b * S
                        WGM = wgmp.tile([EDIM, CNUM, S], FP16, tag="WGM")
                        w_flat = w_r.ap()[b:b + 1].rearrange(
                            "a c t -> a (c t)")
                        nc.sync.dma_start(
                            WGM[:].rearrange("p c t -> p (c t)"),
                            w_flat.to_broadcast([EDIM, CT]))
                        # G = W * e_bv (in place); split c-wise Pool/DVE
                        cp = G_SPLIT[b]
                        e_bv = e16[:, b:b + 1, :].to_broadcast(
                            [EDIM, CNUM, S])
                        if cp > 0:
                            nc.gpsimd.tensor_tensor(
                                out=WGM[:, 0:cp, :], in0=WGM[:, 0:cp, :],
                                in1=e_bv[:, 0:cp, :], op=OP.mult)
                        if cp < CNUM:
                            nc.vector.tensor_tensor(
                                out=WGM[:, cp:CNUM, :],
                                in0=WGM[:, cp:CNUM, :],
                                in1=e_bv[:, cp:CNUM, :], op=OP.mult)
                        # M = 1 - G (ACT affine, in place; DVE half first
                        # so the first scans can start earlier)
                        nc.scalar.activation(WGM[:, cp:CNUM, :],
                                             WGM[:, cp:CNUM, :], AF.Copy,
                                             bias=1.0, scale=-1.0)
                        if cp > 0:
                            nc.scalar.activation(WGM[:, 0:cp, :],
                                                 WGM[:, 0:cp, :], AF.Copy,
                                                 bias=1.0, scale=-1.0)

                        V = vp.tile([EDIM, CNUM, S], FP16, tag="V")
                        corder = list(range(cp, CNUM)) + list(range(cp))
                        for c in corder:
                            nc.vector.tensor_tensor_scan(
                                out=V[:, c, :], data0=WGM[:, c, :],
                                data1=d1t[:, b, :],
                                initial=yinit[:, b, c:c + 1],
                                op0=OP.mult, op1=OP.add)

                        ups = upsp.tile([EDIM, S], F32, tag="ups")
                        for i, c in enumerate(corder):
                            nc.tensor.matmul(ups[:], id_sb[:], V[:, c, :],
                                             start=(i == 0),
                                             stop=(i == CNUM - 1))
                        nc.scalar.copy(Ub[:, b, 1:SP], ups[:])

                        # r = (Uy_{t-1} - Uy_t) * er + f2
                        h = b % 2
                        nc.vector.tensor_tensor(
                            out=rtmp[:, h, :], in0=Ub[:, b, 0:S],
                            in1=Ub[:, b, 1:SP], op=OP.subtract)
                        nc.vector.tensor_tensor(
                            out=rtmp[:, h, :], in0=rtmp[:, h, :],
                            in1=er[:, b, :], op=OP.mult)
                        nc.vector.tensor_tensor(
                            out=r16[:, b, :], in0=rtmp[:, h, :],
                            in1=f2[:, b, :], op=OP.add)

                        # ---- phase 3 per batch ----
                        hps = hpsp.tile([EDIM, S], F32, tag="hps")
                        nc.tensor.matmul(hps[:], lin1_sb[:], r16[:, b, :],
                                         start=True, stop=False)
                        nc.tensor.matmul(hps[:], lin2_sb[:],
                                         itm16[:, bc0:bc0 + S],
                                         start=False, stop=True)
                        h16 = p3.tile([EDIM, S], BF16, tag="h16")
                        nc.scalar.activation(h16[:], hps[:], AF.Tanh,
                                             bias=lin_b_col[:], scale=1.0)
                        for (s0, sw) in ((0, 128), (128, 72)):
                            ot = p3.tile([128, INUM], BF16, tag="ot")
                            for hf in range(2):
                                hc = slice(hf * 500, hf * 500 + 500)
                                ps_o = clsp.tile([128, 500], F32,
                                                 tag="ps_o")
                                nc.tensor.matmul(ps_o[:sw],
                                                 h16[:, s0:s0 + sw],
                                                 cls_w_sb[:, hc],
                                                 start=True, stop=False)
                                nc.tensor.matmul(ps_o[:sw],
                                                 ones16[:1, :sw],
                                                 cls_b_sb[:, hc],
                                                 start=False, stop=True)
                                nc.scalar.activation(ot[:sw, hc],
                                                     ps_o[:sw], AF.Sigmoid)
                            nc.sync.dma_start(
                                out.ap()[bc0 + s0: bc0 + s0 + sw], ot[:sw])

                # staggered emission: chunk q lands two pairs ahead of its
                # phase-2 consumers so w_r / e / d1 are never the blocker
                phase1(0)
                phase1(1)
                phase23(0)
                phase23(1)
                phase1(2)
                phase23(2)
                phase23(3)
                phase1(3)
                phase23(4)
                phase23(5)
                phase23(6)
                phase23(7)

    nc.compile()
    return nc


def kernel(**inputs):
    global LAST_RESULT
    if "nc" not in _NC_CACHE:
        _NC_CACHE["nc"] = _build()
    nc = _NC_CACHE["nc"]

    bf = ml_dtypes.bfloat16
    f32 = {k: np.asarray(inputs[k], dtype=np.float32) for k in inputs}
    lin_w = f32["lin_w"]
    shared = {
        "A_wT": np.ascontiguousarray(f32["A_w"].T).astype(bf),
        "B_wT": np.ascontiguousarray(f32["B_w"].T).astype(bf),
        "kmatT": np.ascontiguousarray(f32["kmat"].T).astype(bf),
        "er_wT": np.ascontiguousarray(f32["er_w"].T).astype(bf),
        "ad_wT": np.ascontiguousarray(f32["ad_w"].T).astype(bf),
        "lin1T": np.ascontiguousarray(lin_w[:, :EDIM].T).astype(bf),
        "lin2T": np.ascontiguousarray(lin_w[:, EDIM:].T).astype(bf),
        "cls_wT": np.ascontiguousarray(f32["cls_w"].T).astype(bf),
        "v0f": np.ascontiguousarray(f32["vmat0"].T),
        "idm": np.eye(EDIM, dtype=np.float32).astype(bf),
        "lin_b": f32["lin_b"],
        "er_b": f32["er_b"],
        "ad_b": f32["ad_b"],
        "cls_b16": np.ascontiguousarray(f32["cls_b"][None, :]).astype(bf),
    }
    item = f32["item"]
    inter = f32["interaction"]

    in_maps = []
    for c in range(NCORES):
        m = dict(shared)
        m["itemT"] = np.ascontiguousarray(
            item[c * BL:(c + 1) * BL].reshape(BT, INUM).T).astype(bf)
        m["interT"] = np.ascontiguousarray(
            inter[c * BL:(c + 1) * BL].reshape(BT, IN_DIM).T).astype(bf)
        in_maps.append(m)

    res = run_bass_kernel_spmd(nc, in_maps, core_ids=list(range(NCORES)))
    LAST_RESULT = res
    outs = [res.results[c]["out"].astype(np.float32).reshape(BL, S, INUM)
            for c in range(NCORES)]
    return np.concatenate(outs, axis=0)


# revision 15
# speedup vs baseline: 1.0253x; 1.0253x over previous
"""DKVMN kernel for Trainium2 (8 NeuronCores, data-parallel over batch).

Shapes (hardcoded): B=64, S=200, INUM=1000, IN_DIM=2000, CNUM=50, EDIM=128.
Per core: B_loc = 8 batches. All engines balanced via two identities:

kappa-substitution: with er = 1/e, kap_t = a_t*er_t, the state y = v - kappa
(kappa_{t-1} := kap_t) follows y_t = (1 - w_t e_t) y_{t-1} + d1_t where
d1_t = kap_t - kap_{t+1} is c-INDEPENDENT -> the scan's additive input is a
shared row; the old A = W*a bulk pass disappears.

Delta-U identity: softmax weights sum to 1 over c, so the read
r_t = sum_c w_t[c] v_{t-1}[c,:] = (U_{t-1} - U_t + a_t) * er_t with
U_t = sum_c v_t = Uy_t + C*kap_{t+1}. So r = (Uy_{t-1}-Uy_t)*er + f2,
f2 = (C*d1)*er + kap. Uy = sum_c y comes from 50 identity-stationary
accumulating PE matmuls (PSUM) -> the old X = W*V pass and the 50-matmul
lin1 reduction disappear.

Per b: w-broadcast DMA -> WGM tile; G = WGM*e_bv (TT, DVE or Pool, in-place);
M = 1-G (ACT affine copy, in-place); 50 per-c scans (DVE, initial=y_init AP);
50 ident matmuls -> Uy (PE); r smalls (DVE); per pair: hps = lin1@r16 +
lin2@itm; h = tanh; out = sigmoid(h@cls_w) in bf16, host upcasts to f32.
"""

import numpy as np
import ml_dtypes

import concourse.bass as bass
import concourse.mybir as mybir
import concourse.tile as tile
from concourse import bacc
from concourse.bass_utils import run_bass_kernel_spmd

F32 = mybir.dt.float32
BF16 = mybir.dt.bfloat16
FP16 = mybir.dt.float16
AF = mybir.ActivationFunctionType
OP = mybir.AluOpType

B, S, INUM, IN_DIM, CNUM, EDIM = 64, 200, 1000, 2000, 50, 128
NCORES = 8
BL = B // NCORES          # 8 batches per core
BT = BL * S               # 1600
IK = 8                    # INUM k-chunks of 125
DK = 16                   # IN_DIM k-chunks of 125
KC = 125
CT = CNUM * S             # 10000
SP = S + 1

# per-b split of the G = W*e multiply: c < G_SPLIT[b] on Pool, rest on
# DVE. Last batch all-DVE to shorten the tail.
G_SPLIT = [40, 40, 40, 40, 40, 40, 38, 20]

_NC_CACHE = {}
LAST_RESULT = None


def _build():
    nc = bacc.Bacc("TRN2", target_bir_lowering=False, debug=False,
                   num_devices=NCORES)

    itemT = nc.dram_tensor("itemT", [INUM, BT], BF16, kind="ExternalInput")
    interT = nc.dram_tensor("interT", [IN_DIM, BT], BF16, kind="ExternalInput")
    A_wT = nc.dram_tensor("A_wT", [INUM, EDIM], BF16, kind="ExternalInput")
    B_wT = nc.dram_tensor("B_wT", [IN_DIM, EDIM], BF16, kind="ExternalInput")
    kmatT = nc.dram_tensor("kmatT", [EDIM, CNUM], BF16, kind="ExternalInput")
    er_wT = nc.dram_tensor("er_wT", [EDIM, EDIM], BF16, kind="ExternalInput")
    ad_wT = nc.dram_tensor("ad_wT", [EDIM, EDIM], BF16, kind="ExternalInput")
    lin1T = nc.dram_tensor("lin1T", [EDIM, EDIM], BF16, kind="ExternalInput")
    lin2T = nc.dram_tensor("lin2T", [EDIM, EDIM], BF16, kind="ExternalInput")
    cls_wT = nc.dram_tensor("cls_wT", [EDIM, INUM], BF16, kind="ExternalInput")
    v0f = nc.dram_tensor("v0f", [EDIM, CNUM], F32, kind="ExternalInput")
    idm = nc.dram_tensor("idm", [EDIM, EDIM], BF16, kind="ExternalInput")
    lin_b = nc.dram_tensor("lin_b", [EDIM], F32, kind="ExternalInput")
    er_b = nc.dram_tensor("er_b", [EDIM], F32, kind="ExternalInput")
    ad_b = nc.dram_tensor("ad_b", [EDIM], F32, kind="ExternalInput")
    cls_b16 = nc.dram_tensor("cls_b16", [1, INUM], BF16, kind="ExternalInput")
    out = nc.dram_tensor("out", [BT, INUM], BF16, kind="ExternalOutput")
    w_r = nc.dram_tensor("w_r", [BL, CNUM, S], FP16, kind="Internal")

    with tile.TileContext(nc) as tc:
        with tc.tile_pool(name="singles", bufs=1) as sg:
            ones16 = sg.tile([1, 128], BF16, tag="ones16")
            nc.vector.memset(ones16[:], 1.0)
            ones50 = sg.tile([CNUM, 1], BF16, tag="ones50")
            nc.vector.memset(ones50[:], 1.0)

            A_w_sb = sg.tile([KC, IK, EDIM], BF16, tag="A_w_sb")
            B_w_sb = sg.tile([KC, DK, EDIM], BF16, tag="B_w_sb")
            kmat_sb = sg.tile([EDIM, CNUM], BF16, tag="kmat_sb")
            er_w_sb = sg.tile([EDIM, EDIM], BF16, tag="er_w_sb")
            ad_w_sb = sg.tile([EDIM, EDIM], BF16, tag="ad_w_sb")
            lin1_sb = sg.tile([EDIM, EDIM], BF16, tag="lin1_sb")
            lin2_sb = sg.tile([EDIM, EDIM], BF16, tag="lin2_sb")
            cls_w_sb = sg.tile([EDIM, INUM], BF16, tag="cls_w_sb")
            v0_sb = sg.tile([EDIM, CNUM], F32, tag="v0_sb")
            id_sb = sg.tile([EDIM, EDIM], BF16, tag="id_sb")
            lin_b_col = sg.tile([EDIM, 1], F32, tag="lin_b_col")
            er_b_col = sg.tile([EDIM, 1], F32, tag="er_b_col")
            ad_b_col = sg.tile([EDIM, 1], F32, tag="ad_b_col")
            cls_b_sb = sg.tile([1, INUM], BF16, tag="cls_b_sb")

            # softmax-critical weights first so chunk 0 starts ASAP
            nc.sync.dma_start(
                A_w_sb[:], A_wT.ap().rearrange("(k p) e -> p k e", p=KC))
            nc.sync.dma_start(kmat_sb[:], kmatT.ap())
            nc.sync.dma_start(
                B_w_sb[:], B_wT.ap().rearrange("(k p) e -> p k e", p=KC))
            nc.sync.dma_start(er_w_sb[:], er_wT.ap())
            nc.sync.dma_start(ad_w_sb[:], ad_wT.ap())
            nc.sync.dma_start(er_b_col[:], er_b.ap()[:, None])
            nc.sync.dma_start(ad_b_col[:], ad_b.ap()[:, None])
            nc.sync.dma_start(v0_sb[:], v0f.ap())
            nc.sync.dma_start(id_sb[:], idm.ap())
            nc.sync.dma_start(lin1_sb[:], lin1T.ap())
            nc.sync.dma_start(lin2_sb[:], lin2T.ap())
            nc.sync.dma_start(cls_w_sb[:], cls_wT.ap())
            nc.sync.dma_start(lin_b_col[:], lin_b.ap()[:, None])
            nc.sync.dma_start(cls_b_sb[:], cls_b16.ap())

            # persistent per-core activations / smalls
            itm16 = sg.tile([EDIM, BT], BF16, tag="itm16")
            e16 = sg.tile([EDIM, BL, S], BF16, tag="e16")
            a16 = sg.tile([EDIM, BL, S], BF16, tag="a16")
            er = sg.tile([EDIM, BL, S], BF16, tag="er")
            kap = sg.tile([EDIM, BL, SP], BF16, tag="kap")
            d1t = sg.tile([EDIM, BL, S], FP16, tag="d1t")
            f2 = sg.tile([EDIM, BL, S], BF16, tag="f2")
            Ub = sg.tile([EDIM, BL, SP], F32, tag="Ub")
            yinit = sg.tile([EDIM, BL, CNUM], F32, tag="yinit")
            r16 = sg.tile([EDIM, BL, S], BF16, tag="r16")
            Sv0 = sg.tile([EDIM, 1], F32, tag="Sv0")
            rtmp = sg.tile([EDIM, 2, S], F32, tag="rtmp")

            nc.vector.memset(kap[:, :, S:SP], 0.0)
            nc.vector.tensor_reduce(out=Sv0[:], in_=v0_sb[:], op=OP.add,
                                    axis=mybir.AxisListType.X)

            with tc.tile_pool(name="p1", bufs=2) as p1, \
                 tc.tile_pool(name="p1w", bufs=2) as p1w, \
                 tc.tile_pool(name="p1ps", bufs=2, space="PSUM") as p1p, \
                 tc.tile_pool(name="wgm", bufs=3) as wgmp, \
                 tc.tile_pool(name="vp", bufs=2) as vp, \
                 tc.tile_pool(name="ups", bufs=2, space="PSUM") as upsp, \
                 tc.tile_pool(name="hps", bufs=2, space="PSUM") as hpsp, \
                 tc.tile_pool(name="clsps", bufs=2, space="PSUM") as clsp, \
                 tc.tile_pool(name="p3", bufs=2) as p3:
                def phase1(q):
                    c0 = q * 400
                    cols = slice(c0, c0 + 400)
                    pr = slice(2 * q, 2 * q + 2)

                    # ---- phase 1, chunk q (2 batches) ----
                    it_ch = p1.tile([KC, IK, 400], BF16, tag="it_ch")
                    nc.sync.dma_start(
                        it_ch[:], itemT.ap()[:, cols].rearrange(
                            "(k p) t -> p k t", p=KC))
                    in_ch = p1.tile([KC, DK, 400], BF16, tag="in_ch")
                    nc.sync.dma_start(
                        in_ch[:], interT.ap()[:, cols].rearrange(
                            "(k p) t -> p k t", p=KC))

                    ps1 = p1p.tile([EDIM, 400], F32, tag="big")
                    for k in range(IK):
                        nc.tensor.matmul(ps1[:], A_w_sb[:, k, :],
                                         it_ch[:, k, :],
                                         start=(k == 0), stop=(k == IK - 1))
                    nc.scalar.copy(itm16[:, cols], ps1[:])

                    # softmax numerator -> w_r immediately (PE/ACT/DMA
                    # only); the normalizer is folded into the e-side lazily
                    ps2 = p1p.tile([EDIM, 400], F32, tag="big")
                    nc.tensor.matmul(ps2[:CNUM], kmat_sb[:], itm16[:, cols],
                                     start=True, stop=True)
                    E_j = p1w.tile([CNUM, 400], BF16, tag="E_j")
                    nc.scalar.activation(E_j[:], ps2[:CNUM], AF.Exp)
                    ps2b = p1p.tile([EDIM, 400], F32, tag="big")
                    nc.tensor.matmul(ps2b[0:1], ones50[:], E_j[:],
                                     start=True, stop=True)
                    zr = p1w.tile([1, 400], F32, tag="zr")
                    nc.vector.reciprocal(zr[:], ps2b[0:1])
                    zr16 = p1w.tile([1, 400], BF16, tag="zr16")
                    nc.scalar.copy(zr16[:], zr[:])
                    ps2c = p1p.tile([EDIM, 400], F32, tag="big")
                    nc.tensor.matmul(ps2c[:CNUM], ones16[:1, :CNUM], zr16[:],
                                     start=True, stop=True)
                    zrbc = p1w.tile([CNUM, 400], BF16, tag="zrbc")
                    nc.scalar.copy(zrbc[:], ps2c[:CNUM])
                    w_j = p1w.tile([CNUM, 400], FP16, tag="w_j")
                    nc.gpsimd.tensor_tensor(out=w_j[:], in0=E_j[:],
                                            in1=zrbc[:], op=OP.mult)
                    nc.sync.dma_start(w_r.ap()[2 * q], w_j[:, 0:S])
                    nc.sync.dma_start(w_r.ap()[2 * q + 1], w_j[:, S:2 * S])

                    ps3 = p1p.tile([EDIM, 400], F32, tag="big")
                    for k in range(DK):
                        nc.tensor.matmul(ps3[:], B_w_sb[:, k, :],
                                         in_ch[:, k, :],
                                         start=(k == 0), stop=(k == DK - 1))
                    itr_j = p1w.tile([EDIM, 400], BF16, tag="itr_j")
                    nc.scalar.copy(itr_j[:], ps3[:])

                    ps4 = p1p.tile([EDIM, 400], F32, tag="big")
                    nc.tensor.matmul(ps4[:], er_w_sb[:], itr_j[:],
                                     start=True, stop=True)
                    nc.scalar.activation(e16[:, pr, :], ps4[:],
                                         AF.Sigmoid, bias=er_b_col[:],
                                         scale=1.0)
                    ps5 = p1p.tile([EDIM, 400], F32, tag="big")
                    nc.tensor.matmul(ps5[:], ad_w_sb[:], itr_j[:],
                                     start=True, stop=True)
                    nc.scalar.activation(a16[:, pr, :], ps5[:],
                                         AF.Tanh, bias=ad_b_col[:], scale=1.0)

                    # kappa smalls: er = 1/e ; kap = a*er ; d1 = kap - kap_+1
                    # f2 = (C*d1)*er + kap
                    with nc.allow_low_precision(reason="er=1/e in bf16 is used self-consistently"):
                        nc.vector.reciprocal(er[:, pr, :], e16[:, pr, :])
                    nc.vector.tensor_tensor(out=kap[:, pr, 0:S],
                                            in0=a16[:, pr, :],
                                            in1=er[:, pr, :], op=OP.mult)
                    nc.vector.tensor_tensor(out=d1t[:, pr, :],
                                            in0=kap[:, pr, 0:S],
                                            in1=kap[:, pr, 1:SP],
                                            op=OP.subtract)
                    nc.vector.scalar_tensor_tensor(
                        out=f2[:, pr, :], in0=d1t[:, pr, :],
                        scalar=float(CNUM), in1=er[:, pr, :],
                        op0=OP.mult, op1=OP.mult)
                    nc.vector.tensor_tensor(out=f2[:, pr, :],
                                            in0=f2[:, pr, :],
                                            in1=kap[:, pr, 0:S], op=OP.add)

                    for b in (2 * q, 2 * q + 1):
                        nc.vector.tensor_tensor(
                            out=yinit[:, b, :], in0=v0_sb[:],
                            in1=kap[:, b, 0:1].to_broadcast([EDIM, CNUM]),
                            op=OP.subtract)
                        nc.vector.scalar_tensor_tensor(
                            out=Ub[:, b, 0:1], in0=kap[:, b, 0:1],
                            scalar=-float(CNUM), in1=Sv0[:],
                            op0=OP.mult, op1=OP.add)

                def phase23(b):
                    if True:
                        bc0 = b * S
                        WGM = wgmp.tile([EDIM, CNUM, S], FP16, tag="WGM")
                        w_flat = w_r.ap()[b:b + 1].rearrange(
                            "a c t -> a (c t)")
                        nc.sync.dma_start(
                            WGM[:].rearrange("p c t -> p (c t)"),
                            w_flat.to_broadcast([EDIM, CT]))
                        # G = W * e_bv (in place); split c-wise Pool/DVE
                        cp = G_SPLIT[b]
                        e_bv = e16[:, b:b + 1, :].to_broadcast(
                            [EDIM, CNUM, S])
                        if cp > 0:
                            nc.gpsimd.tensor_tensor(
                                out=WGM[:, 0:cp, :], in0=WGM[:, 0:cp, :],
                                in1=e_bv[:, 0:cp, :], op=OP.mult)
                        if cp < CNUM:
                            nc.vector.tensor_tensor(
                                out=WGM[:, cp:CNUM, :],
                                in0=WGM[:, cp:CNUM, :],
                                in1=e_bv[:, cp:CNUM, :], op=OP.mult)
                        # M = 1 - G (ACT affine, in place; DVE half first
                        # so the first scans can start earlier)
                        nc.scalar.activation(WGM[:, cp:CNUM, :],
                                             WGM[:, cp:CNUM, :], AF.Copy,
                                             bias=1.0, scale=-1.0)
                        if cp > 0:
                            nc.scalar.activation(WGM[:, 0:cp, :],
                                                 WGM[:, 0:cp, :], AF.Copy,
                                                 bias=1.0, scale=-1.0)

                        V = vp.tile([EDIM, CNUM, S], FP16, tag="V")
                        corder = list(range(cp, CNUM)) + list(range(cp))
                        for c in corder:
                            nc.vector.tensor_tensor_scan(
                                out=V[:, c, :], data0=WGM[:, c, :],
                                data1=d1t[:, b, :],
                                initial=yinit[:, b, c:c + 1],
                                op0=OP.mult, op1=OP.add)

                        ups = upsp.tile([EDIM, S], F32, tag="ups")
                        for i, c in enumerate(corder):
                            nc.tensor.matmul(ups[:], id_sb[:], V[:, c, :],
                                             start=(i == 0),
                                             stop=(i == CNUM - 1))
                        nc.scalar.copy(Ub[:, b, 1:SP], ups[:])

                        # r = (Uy_{t-1} - Uy_t) * er + f2
                        h = b % 2
                        nc.vector.tensor_tensor(
                            out=rtmp[:, h, :], in0=Ub[:, b, 0:S],
                            in1=Ub[:, b, 1:SP], op=OP.subtract)
                        nc.vector.tensor_tensor(
                            out=rtmp[:, h, :], in0=rtmp[:, h, :],
                            in1=er[:, b, :], op=OP.mult)
                        nc.vector.tensor_tensor(
                            out=r16[:, b, :], in0=rtmp[:, h, :],
                            in1=f2[:, b, :], op=OP.add)

                        # ---- phase 3 per batch ----
                        hps = hpsp.tile([EDIM, S], F32, tag="hps")
                        nc.tensor.matmul(hps[:], lin1_sb[:], r16[:, b, :],
                                         start=True, stop=False)
                        nc.tensor.matmul(hps[:], lin2_sb[:],
                                         itm16[:, bc0:bc0 + S],
                                         start=False, stop=True)
                        h16 = p3.tile([EDIM, S], BF16, tag="h16")
                        nc.scalar.activation(h16[:], hps[:], AF.Tanh,
                                             bias=lin_b_col[:], scale=1.0)
                        for (s0, sw) in ((0, 128), (128, 72)):
                            ot = p3.tile([128, INUM], BF16, tag="ot")
                            for hf in range(2):
                                hc = slice(hf * 500, hf * 500 + 500)
                                ps_o = clsp.tile([128, 500], F32,
                                                 tag="ps_o")
                                nc.tensor.matmul(ps_o[:sw],
                                                 h16[:, s0:s0 + sw],
                                                 cls_w_sb[:, hc],
                                                 start=True, stop=False)
                                nc.tensor.matmul(ps_o[:sw],
                                                 ones16[:1, :sw],
                                                 cls_b_sb[:, hc],
                                                 start=False, stop=True)
                                nc.scalar.activation(ot[:sw, hc],
                                                     ps_o[:sw], AF.Sigmoid)
                            nc.sync.dma_start(
                                out.ap()[bc0 + s0: bc0 + s0 + sw], ot[:sw])

                # staggered emission: chunk q lands two pairs ahead of its
                # phase-2 consumers so w_r / e / d1 are never the blocker
                phase1(0)
                phase1(1)
                phase23(0)
                phase1(2)
                phase23(1)
                phase23(2)
                phase1(3)
                phase23(3)
                phase23(4)
                phase23(5)
                phase23(6)
                phase23(7)

    nc.compile()
    return nc


def kernel(**inputs):
    global LAST_RESULT
    if "nc" not in _NC_CACHE:
        _NC_CACHE["nc"] = _build()
    nc = _NC_CACHE["nc"]

    bf = ml_dtypes.bfloat16
    f32 = {k: np.asarray(inputs[k], dtype=np.float32) for k in inputs}
    lin_w = f32["lin_w"]
    shared = {
        "A_wT": np.ascontiguousarray(f32["A_w"].T).astype(bf),
        "B_wT": np.ascontiguousarray(f32["B_w"].T).astype(bf),
        "kmatT": np.ascontiguousarray(f32["kmat"].T).astype(bf),
        "er_wT": np.ascontiguousarray(f32["er_w"].T).astype(bf),
        "ad_wT": np.ascontiguousarray(f32["ad_w"].T).astype(bf),
        "lin1T": np.ascontiguousarray(lin_w[:, :EDIM].T).astype(bf),
        "lin2T": np.ascontiguousarray(lin_w[:, EDIM:].T).astype(bf),
        "cls_wT": np.ascontiguousarray(f32["cls_w"].T).astype(bf),
        "v0f": np.ascontiguousarray(f32["vmat0"].T),
        "idm": np.eye(EDIM, dtype=np.float32).astype(bf),
        "lin_b": f32["lin_b"],
        "er_b": f32["er_b"],
        "ad_b": f32["ad_b"],
        "cls_b16": np.ascontiguousarray(f32["cls_b"][None, :]).astype(bf),
    }
    item = f32["item"]
    inter = f32["interaction"]

    in_maps = []
    for c in range(NCORES):
        m = dict(shared)
        m["itemT"] = np.ascontiguousarray(
            item[c * BL:(c + 1) * BL].reshape(BT, INUM).T).astype(bf)
        m["interT"] = np.ascontiguousarray(
            inter[c * BL:(c + 1) * BL].reshape(BT, IN_DIM).T).astype(bf)
        in_maps.append(m)

    res = run_bass_kernel_spmd(nc, in_maps, core_ids=list(range(NCORES)))
    LAST_RESULT = res
    outs = [res.results[c]["out"].astype(np.float32).reshape(BL, S, INUM)
            for c in range(NCORES)]
    return np.concatenate(outs, axis=0)


# revision 18
# speedup vs baseline: 1.0494x; 1.0235x over previous
"""DKVMN kernel for Trainium2 (8 NeuronCores, data-parallel over batch).

Shapes (hardcoded): B=64, S=200, INUM=1000, IN_DIM=2000, CNUM=50, EDIM=128.
Per core: B_loc = 8 batches. All engines balanced via two identities:

kappa-substitution: with er = 1/e, kap_t = a_t*er_t, the state y = v - kappa
(kappa_{t-1} := kap_t) follows y_t = (1 - w_t e_t) y_{t-1} + d1_t where
d1_t = kap_t - kap_{t+1} is c-INDEPENDENT -> the scan's additive input is a
shared row; the old A = W*a bulk pass disappears.

Delta-U identity: softmax weights sum to 1 over c, so the read
r_t = sum_c w_t[c] v_{t-1}[c,:] = (U_{t-1} - U_t + a_t) * er_t with
U_t = sum_c v_t = Uy_t + C*kap_{t+1}. So r = (Uy_{t-1}-Uy_t)*er + f2,
f2 = (C*d1)*er + kap. Uy = sum_c y comes from 50 identity-stationary
accumulating PE matmuls (PSUM) -> the old X = W*V pass and the 50-matmul
lin1 reduction disappear.

Per b: w-broadcast DMA -> WGM tile; G = WGM*e_bv (TT, DVE or Pool, in-place);
M = 1-G (ACT affine copy, in-place); 50 per-c scans (DVE, initial=y_init AP);
50 ident matmuls -> Uy (PE); r smalls (DVE); per pair: hps = lin1@r16 +
lin2@itm; h = tanh; out = sigmoid(h@cls_w) in bf16, host upcasts to f32.
"""

import numpy as np
import ml_dtypes

import concourse.bass as bass
import concourse.mybir as mybir
import concourse.tile as tile
from concourse import bacc
from concourse.bass_utils import run_bass_kernel_spmd

F32 = mybir.dt.float32
BF16 = mybir.dt.bfloat16
FP16 = mybir.dt.float16
AF = mybir.ActivationFunctionType
OP = mybir.AluOpType

B, S, INUM, IN_DIM, CNUM, EDIM = 64, 200, 1000, 2000, 50, 128
NCORES = 8
BL = B // NCORES          # 8 batches per core
BT = BL * S               # 1600
IK = 8                    # INUM k-chunks of 125
DK = 16                   # IN_DIM k-chunks of 125
KC = 125
CT = CNUM * S             # 10000
SP = S + 1

# per-b split of the G = W*e multiply: c < G_SPLIT[b] on Pool, rest on
# DVE. Last batch all-DVE to shorten the tail.
G_SPLIT = [40, 40, 40, 40, 40, 40, 38, 20]

_NC_CACHE = {}
LAST_RESULT = None


def _build():
    nc = bacc.Bacc("TRN2", target_bir_lowering=False, debug=False,
                   num_devices=NCORES)

    itemT = nc.dram_tensor("itemT", [INUM, BT], BF16, kind="ExternalInput")
    interT = nc.dram_tensor("interT", [IN_DIM, BT], BF16, kind="ExternalInput")
    A_wT = nc.dram_tensor("A_wT", [INUM, EDIM], BF16, kind="ExternalInput")
    B_wT = nc.dram_tensor("B_wT", [IN_DIM, EDIM], BF16, kind="ExternalInput")
    kmatT = nc.dram_tensor("kmatT", [EDIM, CNUM], BF16, kind="ExternalInput")
    er_wT = nc.dram_tensor("er_wT", [EDIM, EDIM], BF16, kind="ExternalInput")
    ad_wT = nc.dram_tensor("ad_wT", [EDIM, EDIM], BF16, kind="ExternalInput")
    lin1T = nc.dram_tensor("lin1T", [EDIM, EDIM], BF16, kind="ExternalInput")
    lin2T = nc.dram_tensor("lin2T", [EDIM, EDIM], BF16, kind="ExternalInput")
    cls_wT = nc.dram_tensor("cls_wT", [EDIM, INUM], BF16, kind="ExternalInput")
    v0f = nc.dram_tensor("v0f", [EDIM, CNUM], F32, kind="ExternalInput")
    idm = nc.dram_tensor("idm", [EDIM, EDIM], BF16, kind="ExternalInput")
    lin_b = nc.dram_tensor("lin_b", [EDIM], F32, kind="ExternalInput")
    er_b = nc.dram_tensor("er_b", [EDIM], F32, kind="ExternalInput")
    ad_b = nc.dram_tensor("ad_b", [EDIM], F32, kind="ExternalInput")
    cls_b16 = nc.dram_tensor("cls_b16", [1, INUM], BF16, kind="ExternalInput")
    out = nc.dram_tensor("out", [BT, INUM], BF16, kind="ExternalOutput")
    w_r = nc.dram_tensor("w_r", [BL, CNUM, S], FP16, kind="Internal")

    with tile.TileContext(nc) as tc:
        with tc.tile_pool(name="singles", bufs=1) as sg:
            ones16 = sg.tile([1, 128], BF16, tag="ones16")
            nc.vector.memset(ones16[:], 1.0)
            ones50 = sg.tile([CNUM, 1], BF16, tag="ones50")
            nc.vector.memset(ones50[:], 1.0)

            A_w_sb = sg.tile([KC, IK, EDIM], BF16, tag="A_w_sb")
            B_w_sb = sg.tile([KC, DK, EDIM], BF16, tag="B_w_sb")
            kmat_sb = sg.tile([EDIM, CNUM], BF16, tag="kmat_sb")
            er_w_sb = sg.tile([EDIM, EDIM], BF16, tag="er_w_sb")
            ad_w_sb = sg.tile([EDIM, EDIM], BF16, tag="ad_w_sb")
            lin1_sb = sg.tile([EDIM, EDIM], BF16, tag="lin1_sb")
            lin2_sb = sg.tile([EDIM, EDIM], BF16, tag="lin2_sb")
            cls_w_sb = sg.tile([EDIM, INUM], BF16, tag="cls_w_sb")
            v0_sb = sg.tile([EDIM, CNUM], F32, tag="v0_sb")
            id_sb = sg.tile([EDIM, EDIM], BF16, tag="id_sb")
            lin_b_col = sg.tile([EDIM, 1], F32, tag="lin_b_col")
            er_b_col = sg.tile([EDIM, 1], F32, tag="er_b_col")
            ad_b_col = sg.tile([EDIM, 1], F32, tag="ad_b_col")
            cls_b_sb = sg.tile([1, INUM], BF16, tag="cls_b_sb")

            # softmax-critical weights first so chunk 0 starts ASAP
            nc.sync.dma_start(
                A_w_sb[:], A_wT.ap().rearrange("(k p) e -> p k e", p=KC))
            nc.sync.dma_start(kmat_sb[:], kmatT.ap())
            nc.sync.dma_start(
                B_w_sb[:], B_wT.ap().rearrange("(k p) e -> p k e", p=KC))
            nc.sync.dma_start(er_w_sb[:], er_wT.ap())
            nc.sync.dma_start(ad_w_sb[:], ad_wT.ap())
            nc.sync.dma_start(er_b_col[:], er_b.ap()[:, None])
            nc.sync.dma_start(ad_b_col[:], ad_b.ap()[:, None])
            nc.sync.dma_start(v0_sb[:], v0f.ap())
            nc.sync.dma_start(id_sb[:], idm.ap())
            nc.sync.dma_start(lin1_sb[:], lin1T.ap())
            nc.sync.dma_start(lin2_sb[:], lin2T.ap())
            nc.sync.dma_start(cls_w_sb[:], cls_wT.ap())
            nc.sync.dma_start(lin_b_col[:], lin_b.ap()[:, None])
            nc.sync.dma_start(cls_b_sb[:], cls_b16.ap())

            # persistent per-core activations / smalls
            itm16 = sg.tile([EDIM, BT], BF16, tag="itm16")
            e16 = sg.tile([EDIM, BL, S], BF16, tag="e16")
            a16 = sg.tile([EDIM, BL, S], BF16, tag="a16")
            er = sg.tile([EDIM, BL, S], BF16, tag="er")
            kap = sg.tile([EDIM, BL, SP], BF16, tag="kap")
            d1t = sg.tile([EDIM, BL, S], FP16, tag="d1t")
            f2 = sg.tile([EDIM, BL, S], BF16, tag="f2")
            Ub = sg.tile([EDIM, BL, SP], F32, tag="Ub")
            yinit = sg.tile([EDIM, BL, CNUM], F32, tag="yinit")
            r16 = sg.tile([EDIM, BL, S], BF16, tag="r16")
            Sv0 = sg.tile([EDIM, 1], F32, tag="Sv0")
            rtmp = sg.tile([EDIM, 2, S], F32, tag="rtmp")

            nc.vector.memset(kap[:, :, S:SP], 0.0)
            nc.vector.tensor_reduce(out=Sv0[:], in_=v0_sb[:], op=OP.add,
                                    axis=mybir.AxisListType.X)

            with tc.tile_pool(name="p1", bufs=2) as p1, \
                 tc.tile_pool(name="p1in", bufs=1) as p1in, \
                 tc.tile_pool(name="p1w", bufs=2) as p1w, \
                 tc.tile_pool(name="p1ps", bufs=2, space="PSUM") as p1p, \
                 tc.tile_pool(name="wgm", bufs=4) as wgmp, \
                 tc.tile_pool(name="vp", bufs=2) as vp, \
                 tc.tile_pool(name="ups", bufs=2, space="PSUM") as upsp, \
                 tc.tile_pool(name="hps", bufs=2, space="PSUM") as hpsp, \
                 tc.tile_pool(name="clsps", bufs=2, space="PSUM") as clsp, \
                 tc.tile_pool(name="p3", bufs=2) as p3:
                def phase1(q):
                    c0 = q * 400
                    cols = slice(c0, c0 + 400)
                    pr = slice(2 * q, 2 * q + 2)

                    # ---- phase 1, chunk q (2 batches) ----
                    it_ch = p1.tile([KC, IK, 400], BF16, tag="it_ch")
                    nc.sync.dma_start(
                        it_ch[:], itemT.ap()[:, cols].rearrange(
                            "(k p) t -> p k t", p=KC))
                    in_ch = p1in.tile([KC, DK, 400], BF16, tag="in_ch")
                    nc.sync.dma_start(
                        in_ch[:], interT.ap()[:, cols].rearrange(
                            "(k p) t -> p k t", p=KC))

                    ps1 = p1p.tile([EDIM, 400], F32, tag="big")
                    for k in range(IK):
                        nc.tensor.matmul(ps1[:], A_w_sb[:, k, :],
                                         it_ch[:, k, :],
                                         start=(k == 0), stop=(k == IK - 1))
                    nc.scalar.copy(itm16[:, cols], ps1[:])

                    # softmax numerator -> w_r immediately (PE/ACT/DMA
                    # only); the normalizer is folded into the e-side lazily
                    ps2 = p1p.tile([EDIM, 400], F32, tag="big")
                    nc.tensor.matmul(ps2[:CNUM], kmat_sb[:], itm16[:, cols],
                                     start=True, stop=True)
                    E_j = p1w.tile([CNUM, 400], BF16, tag="E_j")
                    nc.scalar.activation(E_j[:], ps2[:CNUM], AF.Exp)
                    ps2b = p1p.tile([EDIM, 400], F32, tag="big")
                    nc.tensor.matmul(ps2b[0:1], ones50[:], E_j[:],
                                     start=True, stop=True)
                    zr = p1w.tile([1, 400], F32, tag="zr")
                    nc.vector.reciprocal(zr[:], ps2b[0:1])
                    zr16 = p1w.tile([1, 400], BF16, tag="zr16")
                    nc.scalar.copy(zr16[:], zr[:])
                    ps2c = p1p.tile([EDIM, 400], F32, tag="big")
                    nc.tensor.matmul(ps2c[:CNUM], ones16[:1, :CNUM], zr16[:],
                                     start=True, stop=True)
                    zrbc = p1w.tile([CNUM, 400], BF16, tag="zrbc")
                    nc.scalar.copy(zrbc[:], ps2c[:CNUM])
                    w_j = p1w.tile([CNUM, 400], FP16, tag="w_j")
                    if q < 2:
                        nc.vector.tensor_tensor(out=w_j[:], in0=E_j[:],
                                                in1=zrbc[:], op=OP.mult)
                    else:
                        nc.gpsimd.tensor_tensor(out=w_j[:], in0=E_j[:],
                                                in1=zrbc[:], op=OP.mult)
                    nc.sync.dma_start(w_r.ap()[2 * q], w_j[:, 0:S])
                    nc.sync.dma_start(w_r.ap()[2 * q + 1], w_j[:, S:2 * S])

                    ps3 = p1p.tile([EDIM, 400], F32, tag="big")
                    for k in range(DK):
                        nc.tensor.matmul(ps3[:], B_w_sb[:, k, :],
                                         in_ch[:, k, :],
                                         start=(k == 0), stop=(k == DK - 1))
                    itr_j = p1w.tile([EDIM, 400], BF16, tag="itr_j")
                    nc.scalar.copy(itr_j[:], ps3[:])

                    ps4 = p1p.tile([EDIM, 400], F32, tag="big")
                    nc.tensor.matmul(ps4[:], er_w_sb[:], itr_j[:],
                                     start=True, stop=True)
                    nc.scalar.activation(e16[:, pr, :], ps4[:],
                                         AF.Sigmoid, bias=er_b_col[:],
                                         scale=1.0)
                    ps5 = p1p.tile([EDIM, 400], F32, tag="big")
                    nc.tensor.matmul(ps5[:], ad_w_sb[:], itr_j[:],
                                     start=True, stop=True)
                    nc.scalar.activation(a16[:, pr, :], ps5[:],
                                         AF.Tanh, bias=ad_b_col[:], scale=1.0)

                def phase1s(q):
                    # kappa smalls, emitted late so they never head-of-line
                    # block the scan stream in DVE's in-order queue
                    pr = slice(2 * q, 2 * q + 2)
                    with nc.allow_low_precision(reason="er=1/e in bf16 is used self-consistently"):
                        nc.vector.reciprocal(er[:, pr, :], e16[:, pr, :])
                    nc.vector.tensor_tensor(out=kap[:, pr, 0:S],
                                            in0=a16[:, pr, :],
                                            in1=er[:, pr, :], op=OP.mult)
                    nc.vector.tensor_tensor(out=d1t[:, pr, :],
                                            in0=kap[:, pr, 0:S],
                                            in1=kap[:, pr, 1:SP],
                                            op=OP.subtract)
                    nc.vector.scalar_tensor_tensor(
                        out=f2[:, pr, :], in0=d1t[:, pr, :],
                        scalar=float(CNUM), in1=er[:, pr, :],
                        op0=OP.mult, op1=OP.mult)
                    nc.vector.tensor_tensor(out=f2[:, pr, :],
                                            in0=f2[:, pr, :],
                                            in1=kap[:, pr, 0:S], op=OP.add)

                    for b in (2 * q, 2 * q + 1):
                        nc.vector.tensor_tensor(
                            out=yinit[:, b, :], in0=v0_sb[:],
                            in1=kap[:, b, 0:1].to_broadcast([EDIM, CNUM]),
                            op=OP.subtract)
                        nc.vector.scalar_tensor_tensor(
                            out=Ub[:, b, 0:1], in0=kap[:, b, 0:1],
                            scalar=-float(CNUM), in1=Sv0[:],
                            op0=OP.mult, op1=OP.add)

                def phase23(b):
                    if True:
                        bc0 = b * S
                        WGM = wgmp.tile([EDIM, CNUM, S], FP16, tag="WGM")
                        w_flat = w_r.ap()[b:b + 1].rearrange(
                            "a c t -> a (c t)")
                        nc.sync.dma_start(
                            WGM[:].rearrange("p c t -> p (c t)"),
                            w_flat.to_broadcast([EDIM, CT]))
                        # G = W * e_bv (in place); split c-wise Pool/DVE
                        cp = G_SPLIT[b]
                        e_bv = e16[:, b:b + 1, :].to_broadcast(
                            [EDIM, CNUM, S])
                        if cp > 0:
                            nc.gpsimd.tensor_tensor(
                                out=WGM[:, 0:cp, :], in0=WGM[:, 0:cp, :],
                                in1=e_bv[:, 0:cp, :], op=OP.mult)
                        if cp < CNUM:
                            nc.vector.tensor_tensor(
                                out=WGM[:, cp:CNUM, :],
                                in0=WGM[:, cp:CNUM, :],
                                in1=e_bv[:, cp:CNUM, :], op=OP.mult)
                        # M = 1 - G (ACT affine, in place; DVE half first
                        # so the first scans can start earlier)
                        nc.scalar.activation(WGM[:, cp:CNUM, :],
                                             WGM[:, cp:CNUM, :], AF.Copy,
                                             bias=1.0, scale=-1.0)
                        if cp > 0:
                            nc.scalar.activation(WGM[:, 0:cp, :],
                                                 WGM[:, 0:cp, :], AF.Copy,
                                                 bias=1.0, scale=-1.0)

                        V = vp.tile([EDIM, CNUM, S], FP16, tag="V")
                        corder = list(range(cp, CNUM)) + list(range(cp))
                        for c in corder:
                            nc.vector.tensor_tensor_scan(
                                out=V[:, c, :], data0=WGM[:, c, :],
                                data1=d1t[:, b, :],
                                initial=yinit[:, b, c:c + 1],
                                op0=OP.mult, op1=OP.add)

                        ups = upsp.tile([EDIM, S], F32, tag="ups")
                        for i, c in enumerate(corder):
                            nc.tensor.matmul(ups[:], id_sb[:], V[:, c, :],
                                             start=(i == 0),
                                             stop=(i == CNUM - 1))
                        nc.scalar.copy(Ub[:, b, 1:SP], ups[:])

                        # r = (Uy_{t-1} - Uy_t) * er + f2
                        h = b % 2
                        nc.vector.tensor_tensor(
                            out=rtmp[:, h, :], in0=Ub[:, b, 0:S],
                            in1=Ub[:, b, 1:SP], op=OP.subtract)
                        nc.vector.tensor_tensor(
                            out=rtmp[:, h, :], in0=rtmp[:, h, :],
                            in1=er[:, b, :], op=OP.mult)
                        nc.vector.tensor_tensor(
                            out=r16[:, b, :], in0=rtmp[:, h, :],
                            in1=f2[:, b, :], op=OP.add)

                        # ---- phase 3 per batch ----
                        hps = hpsp.tile([EDIM, S], F32, tag="hps")
                        nc.tensor.matmul(hps[:], lin1_sb[:], r16[:, b, :],
                                         start=True, stop=False)
                        nc.tensor.matmul(hps[:], lin2_sb[:],
                                         itm16[:, bc0:bc0 + S],
                                         start=False, stop=True)
                        h16 = p3.tile([EDIM, S], BF16, tag="h16")
                        nc.scalar.activation(h16[:], hps[:], AF.Tanh,
                                             bias=lin_b_col[:], scale=1.0)
                        for (s0, sw) in ((0, 128), (128, 72)):
                            ot = p3.tile([128, INUM], BF16, tag="ot")
                            for hf in range(2):
                                hc = slice(hf * 500, hf * 500 + 500)
                                ps_o = clsp.tile([128, 500], F32,
                                                 tag="ps_o")
                                nc.tensor.matmul(ps_o[:sw],
                                                 h16[:, s0:s0 + sw],
                                                 cls_w_sb[:, hc],
                                                 start=True, stop=False)
                                nc.tensor.matmul(ps_o[:sw],
                                                 ones16[:1, :sw],
                                                 cls_b_sb[:, hc],
                                                 start=False, stop=True)
                                nc.scalar.activation(ot[:sw, hc],
                                                     ps_o[:sw], AF.Sigmoid)
                            nc.sync.dma_start(
                                out.ap()[bc0 + s0: bc0 + s0 + sw], ot[:sw])

                # staggered emission: chunk q lands two pairs ahead of its
                # phase-2 consumers so w_r / e / d1 are never the blocker
                phase1(0)
                phase1(1)
                phase1s(0)
                phase23(0)
                phase1(2)
                phase23(1)
                phase1s(1)
                phase23(2)
                phase1(3)
                phase23(3)
                phase1s(2)
                phase23(4)
                phase23(5)
                phase1s(3)
                phase23(6)
                phase23(7)

    nc.compile()
    return nc


def kernel(**inputs):
    global LAST_RESULT
    if "nc" not in _NC_CACHE:
        _NC_CACHE["nc"] = _build()
    nc = _NC_CACHE["nc"]

    bf = ml_dtypes.bfloat16
    f32 = {k: np.asarray(inputs[k], dtype=np.float32) for k in inputs}
    lin_w = f32["lin_w"]
    shared = {
        "A_wT": np.ascontiguousarray(f32["A_w"].T).astype(bf),
        "B_wT": np.ascontiguousarray(f32["B_w"].T).astype(bf),
        "kmatT": np.ascontiguousarray(f32["kmat"].T).astype(bf),
        "er_wT": np.ascontiguousarray(f32["er_w"].T).astype(bf),
        "ad_wT": np.ascontiguousarray(f32["ad_w"].T).astype(bf),
        "lin1T": np.ascontiguousarray(lin_w[:, :EDIM].T).astype(bf),
        "lin2T": np.ascontiguousarray(lin_w[:, EDIM:].T).astype(bf),
        "cls_wT": np.ascontiguousarray(f32["cls_w"].T).astype(bf),
        "v0f": np.ascontiguousarray(f32["vmat0"].T),
        "idm": np.eye(EDIM, dtype=np.float32).astype(bf),
        "lin_b": f32["lin_b"],
        "er_b": f32["er_b"],
        "ad_b": f32["ad_b"],
        "cls_b16": np.ascontiguousarray(f32["cls_b"][None, :]).astype(bf),
    }
    item = f32["item"]
    inter = f32["interaction"]

    in_maps = []
    for c in range(NCORES):
        m = dict(shared)
        m["itemT"] = np.ascontiguousarray(
            item[c * BL:(c + 1) * BL].reshape(BT, INUM).T).astype(bf)
        m["interT"] = np.ascontiguousarray(
            inter[c * BL:(c + 1) * BL].reshape(BT, IN_DIM).T).astype(bf)
        in_maps.append(m)

    res = run_bass_kernel_spmd(nc, in_maps, core_ids=list(range(NCORES)))
    LAST_RESULT = res
    outs = [res.results[c]["out"].astype(np.float32).reshape(BL, S, INUM)
            for c in range(NCORES)]
    return np.concatenate(outs, axis=0)


# revision 19
# speedup vs baseline: 1.1020x; 1.0501x over previous
"""DKVMN kernel for Trainium2 (8 NeuronCores, data-parallel over batch).

Shapes (hardcoded): B=64, S=200, INUM=1000, IN_DIM=2000, CNUM=50, EDIM=128.
Per core: B_loc = 8 batches. All engines balanced via two identities:

kappa-substitution: with er = 1/e, kap_t = a_t*er_t, the state y = v - kappa
(kappa_{t-1} := kap_t) follows y_t = (1 - w_t e_t) y_{t-1} + d1_t where
d1_t = kap_t - kap_{t+1} is c-INDEPENDENT -> the scan's additive input is a
shared row; the old A = W*a bulk pass disappears.

Delta-U identity: softmax weights sum to 1 over c, so the read
r_t = sum_c w_t[c] v_{t-1}[c,:] = (U_{t-1} - U_t + a_t) * er_t with
U_t = sum_c v_t = Uy_t + C*kap_{t+1}. So r = (Uy_{t-1}-Uy_t)*er + f2,
f2 = (C*d1)*er + kap. Uy = sum_c y comes from 50 identity-stationary
accumulating PE matmuls (PSUM) -> the old X = W*V pass and the 50-matmul
lin1 reduction disappear.

Per b: w-broadcast DMA -> WGM tile; G = WGM*e_bv (TT, DVE or Pool, in-place);
M = 1-G (ACT affine copy, in-place); 50 per-c scans (DVE, initial=y_init AP);
50 ident matmuls -> Uy (PE); r smalls (DVE); per pair: hps = lin1@r16 +
lin2@itm; h = tanh; out = sigmoid(h@cls_w) in bf16, host upcasts to f32.
"""

import numpy as np
import ml_dtypes

import concourse.bass as bass
import concourse.mybir as mybir
import concourse.tile as tile
from concourse import bacc
from concourse.bass_utils import run_bass_kernel_spmd

F32 = mybir.dt.float32
BF16 = mybir.dt.bfloat16
FP16 = mybir.dt.float16
AF = mybir.ActivationFunctionType
OP = mybir.AluOpType

B, S, INUM, IN_DIM, CNUM, EDIM = 64, 200, 1000, 2000, 50, 128
NCORES = 8
BL = B // NCORES          # 8 batches per core
BT = BL * S               # 1600
IK = 8                    # INUM k-chunks of 125
DK = 16                   # IN_DIM k-chunks of 125
KC = 125
CT = CNUM * S             # 10000
SP = S + 1

# per-b split of the G = W*e multiply: c < G_SPLIT[b] on Pool, rest on
# DVE. Last batch all-DVE to shorten the tail.
G_SPLIT = [40, 40, 40, 40, 40, 40, 36, 20]

_NC_CACHE = {}
LAST_RESULT = None


def _build():
    nc = bacc.Bacc("TRN2", target_bir_lowering=False, debug=False,
                   num_devices=NCORES)

    itemT = nc.dram_tensor("itemT", [INUM, BT], BF16, kind="ExternalInput")
    interT = nc.dram_tensor("interT", [IN_DIM, BT], BF16, kind="ExternalInput")
    A_wT = nc.dram_tensor("A_wT", [INUM, EDIM], BF16, kind="ExternalInput")
    B_wT = nc.dram_tensor("B_wT", [IN_DIM, EDIM], BF16, kind="ExternalInput")
    kmatT = nc.dram_tensor("kmatT", [EDIM, CNUM], BF16, kind="ExternalInput")
    er_wT = nc.dram_tensor("er_wT", [EDIM, EDIM], BF16, kind="ExternalInput")
    ad_wT = nc.dram_tensor("ad_wT", [EDIM, EDIM], BF16, kind="ExternalInput")
    lin1T = nc.dram_tensor("lin1T", [EDIM, EDIM], BF16, kind="ExternalInput")
    lin2T = nc.dram_tensor("lin2T", [EDIM, EDIM], BF16, kind="ExternalInput")
    cls_wT = nc.dram_tensor("cls_wT", [EDIM, INUM], BF16, kind="ExternalInput")
    v0f = nc.dram_tensor("v0f", [EDIM, CNUM], F32, kind="ExternalInput")
    idm = nc.dram_tensor("idm", [EDIM, EDIM], BF16, kind="ExternalInput")
    lin_b = nc.dram_tensor("lin_b", [EDIM], F32, kind="ExternalInput")
    er_b = nc.dram_tensor("er_b", [EDIM], F32, kind="ExternalInput")
    ad_b = nc.dram_tensor("ad_b", [EDIM], F32, kind="ExternalInput")
    cls_b16 = nc.dram_tensor("cls_b16", [1, INUM], BF16, kind="ExternalInput")
    out = nc.dram_tensor("out", [BT, INUM], BF16, kind="ExternalOutput")
    w_r = nc.dram_tensor("w_r", [BL, CNUM, S], FP16, kind="Internal")

    with tile.TileContext(nc) as tc:
        with tc.tile_pool(name="singles", bufs=1) as sg:
            ones16 = sg.tile([1, 128], BF16, tag="ones16")
            nc.vector.memset(ones16[:], 1.0)
            ones50 = sg.tile([CNUM, 1], BF16, tag="ones50")
            nc.vector.memset(ones50[:], 1.0)

            A_w_sb = sg.tile([KC, IK, EDIM], BF16, tag="A_w_sb")
            B_w_sb = sg.tile([KC, DK, EDIM], BF16, tag="B_w_sb")
            kmat_sb = sg.tile([EDIM, CNUM], BF16, tag="kmat_sb")
            er_w_sb = sg.tile([EDIM, EDIM], BF16, tag="er_w_sb")
            ad_w_sb = sg.tile([EDIM, EDIM], BF16, tag="ad_w_sb")
            lin1_sb = sg.tile([EDIM, EDIM], BF16, tag="lin1_sb")
            lin2_sb = sg.tile([EDIM, EDIM], BF16, tag="lin2_sb")
            cls_w_sb = sg.tile([EDIM, INUM], BF16, tag="cls_w_sb")
            v0_sb = sg.tile([EDIM, CNUM], F32, tag="v0_sb")
            id_sb = sg.tile([EDIM, EDIM], BF16, tag="id_sb")
            lin_b_col = sg.tile([EDIM, 1], F32, tag="lin_b_col")
            er_b_col = sg.tile([EDIM, 1], F32, tag="er_b_col")
            ad_b_col = sg.tile([EDIM, 1], F32, tag="ad_b_col")
            cls_b_sb = sg.tile([1, INUM], BF16, tag="cls_b_sb")

            # softmax-critical weights first so chunk 0 starts ASAP;
            # everything else is deferred until after chunk-0 input loads
            nc.sync.dma_start(
                A_w_sb[:], A_wT.ap().rearrange("(k p) e -> p k e", p=KC))
            nc.sync.dma_start(kmat_sb[:], kmatT.ap())

            def late_weight_dmas():
                nc.sync.dma_start(
                    B_w_sb[:], B_wT.ap().rearrange("(k p) e -> p k e", p=KC))
                nc.sync.dma_start(er_w_sb[:], er_wT.ap())
                nc.sync.dma_start(ad_w_sb[:], ad_wT.ap())
                nc.sync.dma_start(er_b_col[:], er_b.ap()[:, None])
                nc.sync.dma_start(ad_b_col[:], ad_b.ap()[:, None])
                nc.sync.dma_start(v0_sb[:], v0f.ap())
                nc.sync.dma_start(id_sb[:], idm.ap())
                nc.sync.dma_start(lin1_sb[:], lin1T.ap())
                nc.sync.dma_start(lin2_sb[:], lin2T.ap())
                nc.sync.dma_start(cls_w_sb[:], cls_wT.ap())
                nc.sync.dma_start(lin_b_col[:], lin_b.ap()[:, None])
                nc.sync.dma_start(cls_b_sb[:], cls_b16.ap())
                nc.vector.tensor_reduce(out=Sv0[:], in_=v0_sb[:], op=OP.add,
                                        axis=mybir.AxisListType.X)

            # persistent per-core activations / smalls
            itm16 = sg.tile([EDIM, BT], BF16, tag="itm16")
            e16 = sg.tile([EDIM, BL, S], BF16, tag="e16")
            a16 = sg.tile([EDIM, BL, S], BF16, tag="a16")
            er = sg.tile([EDIM, BL, S], BF16, tag="er")
            kap = sg.tile([EDIM, BL, SP], BF16, tag="kap")
            d1t = sg.tile([EDIM, BL, S], FP16, tag="d1t")
            f2 = sg.tile([EDIM, BL, S], BF16, tag="f2")
            Ub = sg.tile([EDIM, BL, SP], F32, tag="Ub")
            yinit = sg.tile([EDIM, BL, CNUM], F32, tag="yinit")
            r16 = sg.tile([EDIM, BL, S], BF16, tag="r16")
            Sv0 = sg.tile([EDIM, 1], F32, tag="Sv0")
            rtmp = sg.tile([EDIM, 2, S], F32, tag="rtmp")

            nc.vector.memset(kap[:, :, S:SP], 0.0)

            with tc.tile_pool(name="p1", bufs=2) as p1, \
                 tc.tile_pool(name="p1in", bufs=1) as p1in, \
                 tc.tile_pool(name="p1w", bufs=2) as p1w, \
                 tc.tile_pool(name="p1ps", bufs=2, space="PSUM") as p1p, \
                 tc.tile_pool(name="wgm", bufs=4) as wgmp, \
                 tc.tile_pool(name="vp", bufs=2) as vp, \
                 tc.tile_pool(name="ups", bufs=2, space="PSUM") as upsp, \
                 tc.tile_pool(name="hps", bufs=2, space="PSUM") as hpsp, \
                 tc.tile_pool(name="clsps", bufs=2, space="PSUM") as clsp, \
                 tc.tile_pool(name="p3", bufs=2) as p3:
                loaded = {}

                def phase1_load(q):
                    c0 = q * 400
                    cols = slice(c0, c0 + 400)
                    it_ch = p1.tile([KC, IK, 400], BF16, tag="it_ch")
                    nc.sync.dma_start(
                        it_ch[:], itemT.ap()[:, cols].rearrange(
                            "(k p) t -> p k t", p=KC))
                    in_ch = p1in.tile([KC, DK, 400], BF16, tag="in_ch")
                    nc.sync.dma_start(
                        in_ch[:], interT.ap()[:, cols].rearrange(
                            "(k p) t -> p k t", p=KC))
                    loaded[q] = (it_ch, in_ch)

                def phase1(q):
                    c0 = q * 400
                    cols = slice(c0, c0 + 400)
                    pr = slice(2 * q, 2 * q + 2)
                    it_ch, in_ch = loaded.pop(q)

                    ps1 = p1p.tile([EDIM, 400], F32, tag="big")
                    for k in range(IK):
                        nc.tensor.matmul(ps1[:], A_w_sb[:, k, :],
                                         it_ch[:, k, :],
                                         start=(k == 0), stop=(k == IK - 1))
                    nc.scalar.copy(itm16[:, cols], ps1[:])

                    # softmax numerator -> w_r immediately (PE/ACT/DMA
                    # only); the normalizer is folded into the e-side lazily
                    ps2 = p1p.tile([EDIM, 400], F32, tag="big")
                    nc.tensor.matmul(ps2[:CNUM], kmat_sb[:], itm16[:, cols],
                                     start=True, stop=True)
                    E_j = p1w.tile([CNUM, 400], BF16, tag="E_j")
                    nc.scalar.activation(E_j[:], ps2[:CNUM], AF.Exp)
                    ps2b = p1p.tile([EDIM, 400], F32, tag="big")
                    nc.tensor.matmul(ps2b[0:1], ones50[:], E_j[:],
                                     start=True, stop=True)
                    zr = p1w.tile([1, 400], F32, tag="zr")
                    nc.vector.reciprocal(zr[:], ps2b[0:1])
                    zr16 = p1w.tile([1, 400], BF16, tag="zr16")
                    nc.scalar.copy(zr16[:], zr[:])
                    ps2c = p1p.tile([EDIM, 400], F32, tag="big")
                    nc.tensor.matmul(ps2c[:CNUM], ones16[:1, :CNUM], zr16[:],
                                     start=True, stop=True)
                    zrbc = p1w.tile([CNUM, 400], BF16, tag="zrbc")
                    nc.scalar.copy(zrbc[:], ps2c[:CNUM])
                    w_j = p1w.tile([CNUM, 400], FP16, tag="w_j")
                    if q < 2:
                        nc.vector.tensor_tensor(out=w_j[:], in0=E_j[:],
                                                in1=zrbc[:], op=OP.mult)
                    else:
                        nc.gpsimd.tensor_tensor(out=w_j[:], in0=E_j[:],
                                                in1=zrbc[:], op=OP.mult)
                    nc.sync.dma_start(w_r.ap()[2 * q], w_j[:, 0:S])
                    nc.sync.dma_start(w_r.ap()[2 * q + 1], w_j[:, S:2 * S])

                    ps3 = p1p.tile([EDIM, 400], F32, tag="big")
                    for k in range(DK):
                        nc.tensor.matmul(ps3[:], B_w_sb[:, k, :],
                                         in_ch[:, k, :],
                                         start=(k == 0), stop=(k == DK - 1))
                    itr_j = p1w.tile([EDIM, 400], BF16, tag="itr_j")
                    nc.scalar.copy(itr_j[:], ps3[:])

                    ps4 = p1p.tile([EDIM, 400], F32, tag="big")
                    nc.tensor.matmul(ps4[:], er_w_sb[:], itr_j[:],
                                     start=True, stop=True)
                    nc.scalar.activation(e16[:, pr, :], ps4[:],
                                         AF.Sigmoid, bias=er_b_col[:],
                                         scale=1.0)
                    ps5 = p1p.tile([EDIM, 400], F32, tag="big")
                    nc.tensor.matmul(ps5[:], ad_w_sb[:], itr_j[:],
                                     start=True, stop=True)
                    nc.scalar.activation(a16[:, pr, :], ps5[:],
                                         AF.Tanh, bias=ad_b_col[:], scale=1.0)

                def phase1s(q):
                    # kappa smalls, emitted late so they never head-of-line
                    # block the scan stream in DVE's in-order queue
                    pr = slice(2 * q, 2 * q + 2)
                    with nc.allow_low_precision(reason="er=1/e in bf16 is used self-consistently"):
                        nc.vector.reciprocal(er[:, pr, :], e16[:, pr, :])
                    nc.vector.tensor_tensor(out=kap[:, pr, 0:S],
                                            in0=a16[:, pr, :],
                                            in1=er[:, pr, :], op=OP.mult)
                    nc.vector.tensor_tensor(out=d1t[:, pr, :],
                                            in0=kap[:, pr, 0:S],
                                            in1=kap[:, pr, 1:SP],
                                            op=OP.subtract)
                    nc.vector.scalar_tensor_tensor(
                        out=f2[:, pr, :], in0=d1t[:, pr, :],
                        scalar=float(CNUM), in1=er[:, pr, :],
                        op0=OP.mult, op1=OP.mult)
                    nc.vector.tensor_tensor(out=f2[:, pr, :],
                                            in0=f2[:, pr, :],
                                            in1=kap[:, pr, 0:S], op=OP.add)

                    for b in (2 * q, 2 * q + 1):
                        nc.vector.tensor_tensor(
                            out=yinit[:, b, :], in0=v0_sb[:],
                            in1=kap[:, b, 0:1].to_broadcast([EDIM, CNUM]),
                            op=OP.subtract)
                        nc.vector.scalar_tensor_tensor(
                            out=Ub[:, b, 0:1], in0=kap[:, b, 0:1],
                            scalar=-float(CNUM), in1=Sv0[:],
                            op0=OP.mult, op1=OP.add)

                def phase23(b):
                    if True:
                        bc0 = b * S
                        WGM = wgmp.tile([EDIM, CNUM, S], FP16, tag="WGM")
                        w_flat = w_r.ap()[b:b + 1].rearrange(
                            "a c t -> a (c t)")
                        nc.sync.dma_start(
                            WGM[:].rearrange("p c t -> p (c t)"),
                            w_flat.to_broadcast([EDIM, CT]))
                        # G = W * e_bv (in place); split c-wise Pool/DVE
                        cp = G_SPLIT[b]
                        e_bv = e16[:, b:b + 1, :].to_broadcast(
                            [EDIM, CNUM, S])
                        if cp > 0:
                            nc.gpsimd.tensor_tensor(
                                out=WGM[:, 0:cp, :], in0=WGM[:, 0:cp, :],
                                in1=e_bv[:, 0:cp, :], op=OP.mult)
                        if cp < CNUM:
                            nc.vector.tensor_tensor(
                                out=WGM[:, cp:CNUM, :],
                                in0=WGM[:, cp:CNUM, :],
                                in1=e_bv[:, cp:CNUM, :], op=OP.mult)
                        # M = 1 - G (ACT affine, in place; DVE half first
                        # so the first scans can start earlier)
                        nc.scalar.activation(WGM[:, cp:CNUM, :],
                                             WGM[:, cp:CNUM, :], AF.Copy,
                                             bias=1.0, scale=-1.0)
                        if cp > 0:
                            nc.scalar.activation(WGM[:, 0:cp, :],
                                                 WGM[:, 0:cp, :], AF.Copy,
                                                 bias=1.0, scale=-1.0)

                        V = vp.tile([EDIM, CNUM, S], FP16, tag="V")
                        corder = list(range(cp, CNUM)) + list(range(cp))
                        for c in corder:
                            nc.vector.tensor_tensor_scan(
                                out=V[:, c, :], data0=WGM[:, c, :],
                                data1=d1t[:, b, :],
                                initial=yinit[:, b, c:c + 1],
                                op0=OP.mult, op1=OP.add)

                        ups = upsp.tile([EDIM, S], F32, tag="ups")
                        for i, c in enumerate(corder):
                            nc.tensor.matmul(ups[:], id_sb[:], V[:, c, :],
                                             start=(i == 0),
                                             stop=(i == CNUM - 1))
                        nc.scalar.copy(Ub[:, b, 1:SP], ups[:])

                        # r = (Uy_{t-1} - Uy_t) * er + f2
                        h = b % 2
                        nc.vector.tensor_tensor(
                            out=rtmp[:, h, :], in0=Ub[:, b, 0:S],
                            in1=Ub[:, b, 1:SP], op=OP.subtract)
                        nc.vector.tensor_tensor(
                            out=rtmp[:, h, :], in0=rtmp[:, h, :],
                            in1=er[:, b, :], op=OP.mult)
                        nc.vector.tensor_tensor(
                            out=r16[:, b, :], in0=rtmp[:, h, :],
                            in1=f2[:, b, :], op=OP.add)

                        # ---- phase 3 per batch ----
                        hps = hpsp.tile([EDIM, S], F32, tag="hps")
                        nc.tensor.matmul(hps[:], lin1_sb[:], r16[:, b, :],
                                         start=True, stop=False)
                        nc.tensor.matmul(hps[:], lin2_sb[:],
                                         itm16[:, bc0:bc0 + S],
                                         start=False, stop=True)
                        h16 = p3.tile([EDIM, S], BF16, tag="h16")
                        nc.scalar.activation(h16[:], hps[:], AF.Tanh,
                                             bias=lin_b_col[:], scale=1.0)
                        for (s0, sw) in ((0, 128), (128, 72)):
                            ot = p3.tile([128, INUM], BF16, tag="ot")
                            for hf in range(2):
                                hc = slice(hf * 500, hf * 500 + 500)
                                ps_o = clsp.tile([128, 500], F32,
                                                 tag="ps_o")
                                nc.tensor.matmul(ps_o[:sw],
                                                 h16[:, s0:s0 + sw],
                                                 cls_w_sb[:, hc],
                                                 start=True, stop=False)
                                nc.tensor.matmul(ps_o[:sw],
                                                 ones16[:1, :sw],
                                                 cls_b_sb[:, hc],
                                                 start=False, stop=True)
                                nc.scalar.activation(ot[:sw, hc],
                                                     ps_o[:sw], AF.Sigmoid)
                            nc.scalar.dma_start(
                                out.ap()[bc0 + s0: bc0 + s0 + sw], ot[:sw])

                # staggered emission: chunk q lands two pairs ahead of its
                # phase-2 consumers so w_r / e / d1 are never the blocker
                phase1_load(0)
                late_weight_dmas()
                phase1(0)
                phase1_load(1)
                phase1(1)
                phase1s(0)
                phase23(0)
                phase1_load(2)
                phase1(2)
                phase23(1)
                phase1s(1)
                phase23(2)
                phase1_load(3)
                phase1(3)
                phase23(3)
                phase1s(2)
                phase23(4)
                phase23(5)
                phase1s(3)
                phase23(6)
                phase23(7)

    nc.compile()
    return nc


def kernel(**inputs):
    global LAST_RESULT
    if "nc" not in _NC_CACHE:
        _NC_CACHE["nc"] = _build()
    nc = _NC_CACHE["nc"]

    bf = ml_dtypes.bfloat16
    f32 = {k: np.asarray(inputs[k], dtype=np.float32) for k in inputs}
    lin_w = f32["lin_w"]
    shared = {
        "A_wT": np.ascontiguousarray(f32["A_w"].T).astype(bf),
        "B_wT": np.ascontiguousarray(f32["B_w"].T).astype(bf),
        "kmatT": np.ascontiguousarray(f32["kmat"].T).astype(bf),
        "er_wT": np.ascontiguousarray(f32["er_w"].T).astype(bf),
        "ad_wT": np.ascontiguousarray(f32["ad_w"].T).astype(bf),
        "lin1T": np.ascontiguousarray(lin_w[:, :EDIM].T).astype(bf),
        "lin2T": np.ascontiguousarray(lin_w[:, EDIM:].T).astype(bf),
        "cls_wT": np.ascontiguousarray(f32["cls_w"].T).astype(bf),
        "v0f": np.ascontiguousarray(f32["vmat0"].T),
        "idm": np.eye(EDIM, dtype=np.float32).astype(bf),
        "lin_b": f32["lin_b"],
        "er_b": f32["er_b"],
        "ad_b": f32["ad_b"],
        "cls_b16": np.ascontiguousarray(f32["cls_b"][None, :]).astype(bf),
    }
    item = f32["item"]
    inter = f32["interaction"]

    in_maps = []
    for c in range(NCORES):
        m = dict(shared)
        m["itemT"] = np.ascontiguousarray(
            item[c * BL:(c + 1) * BL].reshape(BT, INUM).T).astype(bf)
        m["interT"] = np.ascontiguousarray(
            inter[c * BL:(c + 1) * BL].reshape(BT, IN_DIM).T).astype(bf)
        in_maps.append(m)

    res = run_bass_kernel_spmd(nc, in_maps, core_ids=list(range(NCORES)))
    LAST_RESULT = res
    outs = [res.results[c]["out"].astype(np.float32).reshape(BL, S, INUM)
            for c in range(NCORES)]
    return np.concatenate(outs, axis=0)
